# revision 23
# baseline (speedup 1.0000x reference)
"""Trainium2 Bass kernel for nn_BaseCompressor2 (truncated one-pole IIR compressor).

Algorithm (per batch n, L=262144, C=2):
  energy[t] = mean_c(sig[c,t]^2); y = IIR_alpha(energy); x = ln(y+1e-5)
  piecewise knee gain; out = exp(log_gain) * sig

Mapping: batch N=32 sharded 4-per-core across 8 cores. Per batch one
[128 x 2048] tile, partition p = time block. The kernel is HBM-bound
(16.8MB/core at ~360GB/s ~ 47us); compute is squeezed under that roofline
on Scalar+Vector only -- GpSimd tensor ops are avoided entirely because Q7
streaming steals SBUF bank ports and inflates every other engine 20-140%
(measured). Signals move through SBUF as bf16 via SWDGE cast-DMAs (HBM side
stays f32), which puts the two output muls in DVE's 2x bf16 tensor_tensor
mode and halves signal SBUF footprint.

Per batch:
  Scalar: Square(s0), Square(s1) -> bf16; Ln -> bf16; u = Relu(x+UK) -> bf16;
          Exp(-c2*p) -> bf16   (five 1x activations, one table set)
  Vector: e = sq0+sq1 (bf16 tt 2x); y = scan(alpha, e) f32 state/out;
          carry stt over T0 cols; m = min(u,knee) (4x); t1 = 2u-knee (4x);
          w = max(t1, u) (2x); p = m*w (2x); out muls g*s16 (2x)
  PE    : cross-partition carry C[p] = y[p-1, -1] via shift-matmul

Knee rewrite (mask-free): q = x + knee/2 - thr, u = relu(q), m = min(u,knee),
w = 2u - m = max(2u - knee, u), -log_gain = c2*m*w, c2 = -c1/(2(knee+1e-3)),
c1 = 1/(ratio+1e-3)-1. (u<=knee: c2 u^2; u>knee: c2 knee(2u-knee); matches
the reference piecewise exactly.)

Carry horizon T0 truncates at alpha^T0 ~ e^-30 (correction < 1e-11 vs the
+1e-5 log floor). bf16 is numerically safe: y >= (1-alpha)/2 * s[t]^2 floors
log-energy wherever |s| is large, so bf16 relative error never lands on
large-|out| elements.

Scalar activation tables: Square/Ln/Relu/Exp all live in
'natural_log_exp_and_others', but the stock greedy chooser bounces between
exp_and_others/natural_log (~2 reloads/batch). A Bacc subclass empties every
other set in the list handed to the insertion pass (indices preserved) so all
activations resolve to that one set -> exactly one ACT_TABLE_LOAD.
"""

import numpy as np

N, C, L = 32, 2, 262144
NCORES = 8
BPC = N // NCORES  # batches per core
P = 128
FD = L // P  # 2048 free elems per partition
H = FD // 2

# f32 param columns (per batch b, base b*NP)
NP = 6
ALPHA, SQC, UK, KNEE, EC2, K2 = range(NP)

_cache = {}


def _host_params(z_alpha_pre, log_threshold, log_ratio, log_knee):
    """Per-batch derived scalars, float64 math -> float32 columns."""
    z = z_alpha_pre.astype(np.float64).reshape(-1)
    thr = log_threshold.astype(np.float64).reshape(-1) - 6.0
    knee = np.exp(log_knee.astype(np.float64).reshape(-1))
    r001 = 1.0 + np.exp(log_ratio.astype(np.float64).reshape(-1)) + 0.001
    alpha = 1.0 / (1.0 + np.exp(-z))
    # consistency: the scan uses f32 alpha; derive the input scale from it
    a32 = alpha.astype(np.float32).astype(np.float64)
    c1 = 1.0 / r001 - 1.0  # < 0
    c2 = -c1 / (2.0 * (knee + 0.001))  # > 0
    v = np.zeros((N, NP), dtype=np.float64)
    v[:, ALPHA] = a32
    v[:, SQC] = np.sqrt((1.0 - a32) / 2.0)
    v[:, UK] = knee / 2.0 - thr
    v[:, KNEE] = knee
    v[:, EC2] = -c2
    v[:, K2] = knee  # scalar2 slot for the t1 = 2u - knee tensor_scalar
    # carry influence horizon: alpha^(t+1)*C < ~1e-11 for t >= T0
    t0 = int(np.ceil(30.0 / max(1e-9, -np.max(np.log(a32))))) + 64
    t0 = min(FD, max(128, (t0 + 63) // 64 * 64))
    return v.astype(np.float32), t0


def _shift_matrix():
    # lhsT[k, m] = 1 iff m == k+1, so (lhsT.T @ f)[m] = f[m-1], row 0 -> 0
    m = np.zeros((P, P), dtype=np.float32)
    m[np.arange(P - 1), np.arange(1, P)] = 1.0
    return m


def _make_bacc(**kw):
    """Bacc whose act-table insertion only sees natural_log_exp_and_others
    (other sets emptied, list order/indices preserved). Falls back to the
    stock pass if internals differ."""
    import concourse.bacc as bacc

    try:
        from concourse.hw_specs import get_activation_tables
        import bass_rust as _bass_rust
        from concourse import mybir

        class _Bacc(bacc.Bacc):
            def insert_act_table_loads(self):
                has_activation = any(
                    isinstance(i, mybir.InstActivation)
                    for b in self.main_func.blocks
                    for i in b.instructions
                )
                if not has_activation:
                    return
                keep = "natural_log_exp_and_others"
                tables = [
                    (k, v if k == keep else set())
                    for k, v in get_activation_tables(self.m.arch).items()
                ]
                assert any(k == keep and v for k, v in tables)
                _bass_rust.insert_act_table_loads(self, tables)

        return _Bacc(**kw)
    except Exception:
        return bacc.Bacc(**kw)


def _build_program(T0):
    from contextlib import ExitStack

    import concourse.bass as bass
    import concourse.tile as tile
    from concourse import mybir

    f32 = mybir.dt.float32
    bf16 = mybir.dt.bfloat16
    Alu = mybir.AluOpType
    Af = mybir.ActivationFunctionType

    nc = _make_bacc(
        trn_type="TRN2", target_bir_lowering=False, debug=False,
        enable_asserts=False, num_devices=NCORES,
    )
    sig = nc.dram_tensor("sig", [BPC, C, L], f32, kind="ExternalInput")
    pcols = nc.dram_tensor("pcols", [P, BPC * NP], f32, kind="ExternalInput")
    shiftm = nc.dram_tensor("shiftm", [P, P], f32, kind="ExternalInput")
    pwt = nc.dram_tensor("pwt", [BPC, T0], f32, kind="ExternalInput")
    out = nc.dram_tensor("out", [BPC, C, L], f32, kind="ExternalOutput")

    with tile.TileContext(nc) as tc, ExitStack() as ctx:
        const = ctx.enter_context(tc.tile_pool(name="const", bufs=1))
        io = ctx.enter_context(tc.tile_pool(name="io", bufs=1))
        wk = ctx.enter_context(tc.tile_pool(name="wk", bufs=2))
        psum = ctx.enter_context(tc.tile_pool(name="psum", bufs=2, space="PSUM"))

        pc = const.tile([P, BPC * NP], f32, tag="pc")
        nc.sync.dma_start(pc, pcols.ap())
        shift_sb = const.tile([P, P], f32, tag="shift")
        nc.sync.dma_start(shift_sb, shiftm.ap())
        zcol = const.tile([P, 1], f32, tag="zcol")
        nc.vector.memset(zcol, 0.0)
        epscol = const.tile([P, 1], f32, tag="epscol")
        nc.vector.memset(epscol, 1e-5)
        # dummy 1-col activation: forces the ACT_TABLE_LOAD to enqueue its
        # TDRAM fetch before the input DMAs hog the rings (else the first
        # Square waits ~10us for the table)
        dummy = const.tile([P, 1], f32, tag="dummy")
        nc.scalar.activation(dummy, zcol, Af.Exp, bias=epscol[:, 0:1])

        # all inputs ride one SWDGE ring as f32->bf16 casts, batch 0 first
        # in halves (a second ring would steal SDMA packet slots from b0's
        # loads and delay the pipeline head). pw rides after b1's inputs
        # (needed only at b0's carry).
        sigs = []
        for b in range(BPC):
            s0 = io.tile([P, FD], bf16, tag=f"s0_{b}")
            s1 = io.tile([P, FD], bf16, tag=f"s1_{b}")
            sigs.append((s0, s1))
        pw_all = const.tile([P, BPC, T0], f32, tag="pw")

        def col(b, j):
            return pc[:, b * NP + j: b * NP + j + 1]

        def issue_in(b):
            s0, s1 = sigs[b]
            d0 = sig.ap()[b, 0].rearrange("(p f) -> p f", p=P)
            d1 = sig.ap()[b, 1].rearrange("(p f) -> p f", p=P)
            if b == 0:
                for j in (0, H):
                    nc.gpsimd.dma_start(s0[:, j:j + H], d0[:, j:j + H])
                    nc.gpsimd.dma_start(s1[:, j:j + H], d1[:, j:j + H])
            else:
                nc.gpsimd.dma_start(s0, d0)
                nc.gpsimd.dma_start(s1, d1)

        def energy_s(b):
            # squares on Scalar (bf16 out regardless of input dtype)
            s0, s1 = sigs[b]
            sq0 = wk.tile([P, FD], bf16, tag="sq0")
            sq1 = wk.tile([P, FD], bf16, tag="sq1")
            chunks = ((0, H), (H, FD)) if b == 0 else ((0, FD),)
            for j0, j1 in chunks:
                nc.scalar.activation(sq0[:, j0:j1], s0[:, j0:j1], Af.Square,
                                     scale=col(b, SQC), bias=zcol[:, 0:1])
                nc.scalar.activation(sq1[:, j0:j1], s1[:, j0:j1], Af.Square,
                                     scale=col(b, SQC), bias=zcol[:, 0:1])
            return sq0, sq1

        def energy_v(b, sq0, sq1):
            e = wk.tile([P, FD], bf16, tag="e")
            chunks = ((0, H), (H, FD)) if b == 0 else ((0, FD),)
            for j0, j1 in chunks:
                nc.vector.tensor_add(e[:, j0:j1], sq0[:, j0:j1],
                                     sq1[:, j0:j1])
            return e

        def stage_scan(b, e):
            # IIR via DVE scan (f32 state/out; data0 = per-partition alpha);
            # then cross-partition carry via PE shift-matmul + stt on T0 cols
            y = wk.tile([P, FD], f32, tag="y")
            chunks = ((0, H), (H, FD)) if b == 0 else ((0, FD),)
            for j0, j1 in chunks:
                init = 0.0 if j0 == 0 else y[:, j0 - 1: j0]
                nc.vector.tensor_tensor_scan(
                    y[:, j0:j1], col(b, ALPHA).to_broadcast((P, j1 - j0)),
                    e[:, j0:j1], init, Alu.mult, Alu.add)
            c_ps = psum.tile([P, 1], f32, tag="C")
            nc.tensor.matmul(c_ps, shift_sb, y[:, FD - 1: FD],
                             start=True, stop=True)
            nc.vector.scalar_tensor_tensor(y[:, 0:T0], pw_all[:, b, :],
                                           c_ps[:, 0:1], y[:, 0:T0],
                                           Alu.mult, Alu.add)
            return y

        def knee_s(b, y):
            # x = ln(y+eps), u = relu(x+UK) on Scalar (bf16 out)
            x = wk.tile([P, FD], bf16, tag="x")
            u = wk.tile([P, FD], bf16, tag="u")
            nc.scalar.activation(x, y, Af.Ln, bias=epscol[:, 0:1])
            nc.scalar.activation(u, x, Af.Relu, bias=col(b, UK))
            return u

        def knee_v(b, u):
            # m = min(u,knee) [4x]; t1 = 2u-knee [4x]; w = max(t1,u) [2x];
            # p = m*w [2x]  (all bf16 on DVE)
            m = wk.tile([P, FD], bf16, tag="m")
            t1 = wk.tile([P, FD], bf16, tag="t1")
            w = wk.tile([P, FD], bf16, tag="w")
            p = wk.tile([P, FD], bf16, tag="p")
            nc.vector.tensor_scalar(m, u, col(b, KNEE), None, Alu.min)
            nc.vector.tensor_scalar(t1, u, 2.0, col(b, K2), Alu.mult,
                                    Alu.subtract)
            nc.vector.tensor_tensor(w, t1, u, Alu.max)
            nc.vector.tensor_tensor(p, m, w, Alu.mult)
            return p

        def stage_out(b, p):
            # gain on Scalar, output muls on DVE (bf16 tt 2x),
            # bf16->f32 cast DMA out. Last batch in halves: shorter tail.
            s0, s1 = sigs[b]
            g = wk.tile([P, FD], bf16, tag="g")
            o0 = wk.tile([P, FD], bf16, tag="o0")
            o1 = wk.tile([P, FD], bf16, tag="o1")
            d0 = out.ap()[b, 0].rearrange("(p f) -> p f", p=P)
            d1 = out.ap()[b, 1].rearrange("(p f) -> p f", p=P)
            chunks = ((0, H), (H, FD)) if b == BPC - 1 else ((0, FD),)
            for j0, j1 in chunks:
                nc.scalar.activation(g[:, j0:j1], p[:, j0:j1], Af.Exp,
                                     scale=col(b, EC2), bias=zcol[:, 0:1])
                nc.vector.tensor_tensor(o0[:, j0:j1], g[:, j0:j1],
                                        s0[:, j0:j1], Alu.mult)
                nc.gpsimd.dma_start(d0[:, j0:j1], o0[:, j0:j1])
                nc.vector.tensor_tensor(o1[:, j0:j1], g[:, j0:j1],
                                        s1[:, j0:j1], Alu.mult)
                nc.gpsimd.dma_start(d1[:, j0:j1], o1[:, j0:j1])

        # software-pipelined issue order: each engine executes its stream in
        # issue order, so S gets [sq0 sq1 sq2 Ln0 u0 sq3 Ln1 u1 Exp0 ...]
        # and V gets [a0 s0 a1 s1 a2 s2 k0 a3 s3 k1 m0 k2 m1 k3 m2 m3] --
        # scans hoisted so the last batch's chain starts as early as the
        # input stream allows, early outputs absorb the DMA slack.
        issue_in(0)
        issue_in(1)
        nc.gpsimd.dma_start(pw_all, bass.AP(pwt, 0, [[0, P], [T0, BPC], [1, T0]]))
        sqs = [None] * BPC
        es = [None] * BPC
        ys = [None] * BPC
        us = [None] * BPC
        ps = [None] * BPC
        sqs[0] = energy_s(0)
        es[0] = energy_v(0, *sqs[0])
        ys[0] = stage_scan(0, es[0])
        sqs[1] = energy_s(1)
        issue_in(2)
        es[1] = energy_v(1, *sqs[1])
        ys[1] = stage_scan(1, es[1])
        sqs[2] = energy_s(2)
        issue_in(3)
        es[2] = energy_v(2, *sqs[2])
        ys[2] = stage_scan(2, es[2])
        us[0] = knee_s(0, ys[0])
        ps[0] = knee_v(0, us[0])
        sqs[3] = energy_s(3)
        us[1] = knee_s(1, ys[1])
        es[3] = energy_v(3, *sqs[3])
        ys[3] = stage_scan(3, es[3])
        ps[1] = knee_v(1, us[1])
        stage_out(0, ps[0])
        us[2] = knee_s(2, ys[2])
        ps[2] = knee_v(2, us[2])
        stage_out(1, ps[1])
        us[3] = knee_s(3, ys[3])
        ps[3] = knee_v(3, us[3])
        stage_out(2, ps[2])
        stage_out(3, ps[3])

    nc.compile()
    return nc


def _get_program(T0):
    key = ("nc", T0)
    if key not in _cache:
        _cache[key] = _build_program(T0)
    return _cache[key]


def _run(inputs, trace=False):
    from concourse.bass_utils import run_bass_kernel_spmd

    sig_full = np.ascontiguousarray(np.asarray(inputs["input_signals"], np.float32))
    pv, T0 = _host_params(
        np.asarray(inputs["z_alpha_pre"], np.float32),
        np.asarray(inputs["log_threshold"], np.float32),
        np.asarray(inputs["log_ratio"], np.float32),
        np.asarray(inputs["log_knee"], np.float32),
    )

    nc = _get_program(T0)
    shm = _shift_matrix()
    zf = np.asarray(inputs["z_alpha_pre"], np.float64).reshape(-1)
    alpha64 = (1.0 / (1.0 + np.exp(-zf))).astype(np.float32).astype(np.float64)
    tpow = np.arange(1, T0 + 1, dtype=np.float64)
    pw_np = np.exp(tpow[None, :] * np.log(alpha64)[:, None]).astype(np.float32)
    in_maps = []
    for k in range(NCORES):
        shard = np.ascontiguousarray(sig_full[k * BPC:(k + 1) * BPC])
        cols = np.broadcast_to(
            pv[k * BPC:(k + 1) * BPC].reshape(1, BPC * NP), (P, BPC * NP)
        )
        in_maps.append({
            "sig": shard,
            "pcols": np.ascontiguousarray(cols),
            "shiftm": shm,
            "pwt": np.ascontiguousarray(pw_np[k * BPC:(k + 1) * BPC]),
        })

    res = run_bass_kernel_spmd(
        nc, in_maps, core_ids=list(range(NCORES)), trace=trace,
    )
    out = np.empty((N, C, L), dtype=np.float32)
    for k in range(NCORES):
        out[k * BPC:(k + 1) * BPC] = res.results[k]["out"]
    return out, res


def kernel(**inputs) -> np.ndarray:
    out, _ = _run(inputs, trace=False)
    return out
